# revision 11
# baseline (speedup 1.0000x reference)
"""Trainium2 Bass kernel for a 4-network attention-weighted feature-map blend.

Math (per batch b):
    attn[b, n] = (q[b] / T) . k[b, n]                 (N=4, d=512)
    w = softmax(attn[b, :])
    res[b, c, y, x] = sum_n w[n] * v[b, n, c, y, x]   (C=64, 256x256)

Outputs: (res [4,1,64,256,256] f32, attn [4,4] f32)

Sharding: 8 cores = 4 batches x 2 halves of the C dimension. Each core gets
q/k for its batch (replicated) and a [4, 32, 256, 256] slice of v, flattened
to [4, 2097152]. The big work (weighted sum) is memory bound: 32 MiB in +
8 MiB out per core.

Per-core device program:
  - attn = q.k via 4 PSUM-accumulated matmuls (d split into 4x128 chunks)
  - softmax on one partition (reduce_max/sub/exp/reduce_sum/recip/mul)
  - broadcast w [1,4] -> [128,4] via K=1 outer-product matmul with ones
  - for each [128, 2048] tile: out = v0*w0; out = (vn*wn) + out  (DVE
    scalar_tensor_tensor chain), accumulating into a [128, 16384] out tile
  - one 8 MiB DMA of the out tile back to DRAM
"""

import numpy as np

TEMPERATURE = 22.627416997969522  # sqrt(512)

B = 4
N = 4
C = 64
HH = 256
WW = 256
D = 512

N_CORES = 8
CSH = C // 2  # 32 channels per core
FLAT = CSH * HH * WW  # 2097152 elements per (core, n)
P = 128
FREE = FLAT // P  # 16384 f32 per partition
CH = 2048  # chunk width (free dim) per DVE op / input DMA
NCHUNK = FREE // CH

_CACHE = {}


def _build_nc():
    import concourse.bacc as bacc
    import concourse.mybir as mybir
    from concourse import tile

    f32 = mybir.dt.float32

    nc = bacc.Bacc(
        "TRN2",
        target_bir_lowering=False,
        debug=False,
        enable_asserts=False,
        num_devices=N_CORES,
    )

    q_d = nc.dram_tensor("q", [D], f32, kind="ExternalInput").ap()
    k_d = nc.dram_tensor("k", [N, D], f32, kind="ExternalInput").ap()
    v_d = nc.dram_tensor("v", [N, FLAT], f32, kind="ExternalInput").ap()
    res_d = nc.dram_tensor("res", [FLAT], f32, kind="ExternalOutput").ap()
    attn_d = nc.dram_tensor("attn", [N], f32, kind="ExternalOutput").ap()

    with tile.TileContext(nc) as tc:
        with (
            tc.tile_pool(name="small", bufs=1) as spool,
            tc.tile_pool(name="psum", bufs=1, space="PSUM") as ppool,
            tc.tile_pool(name="vin", bufs=8) as vpool,
            tc.tile_pool(name="tmp", bufs=3) as tpool,
            tc.tile_pool(name="outp", bufs=1) as opool,
        ):
            # ---- attn = (q/T) . k, then softmax -> w, broadcast to 128 parts
            # Tiny transposed loads go first on the sync ring (~1us of small
            # packets) so the softmax weights are ready early; fat v reads
            # stream right behind them.
            v_r = v_d.rearrange("n (p f) -> n p f", p=P)
            q_t = spool.tile([P, 4], f32)  # q_t[p, i] = q[i*128 + p]
            nc.sync.dma_start(out=q_t[:], in_=q_d.rearrange("(i p) -> p i", p=P))
            k_t = spool.tile([P, 4, 4], f32)  # k_t[p, i, n] = k[n, i*128 + p]
            for i in range(4):
                nc.sync.dma_start(
                    out=k_t[:, i, :],
                    in_=k_d[:, i * P : (i + 1) * P].rearrange("n p -> p n"),
                )

            attn_ps = ppool.tile([1, 4], f32)
            for i in range(4):
                nc.tensor.matmul(
                    attn_ps[:],
                    lhsT=q_t[:, i : i + 1],
                    rhs=k_t[:, i, :],
                    start=(i == 0),
                    stop=(i == 3),
                )

            attn_sb = spool.tile([1, 4], f32)
            nc.scalar.mul(attn_sb[:], attn_ps[:], 1.0 / TEMPERATURE)
            nc.gpsimd.dma_start(
                out=attn_d.rearrange("(p f) -> p f", p=1), in_=attn_sb[:]
            )

            # ---- fat v reads for the first chunks start here
            head_tiles = {}
            for j in range(2):
                sl = slice(j * CH, (j + 1) * CH)
                for n in range(N):
                    vt = vpool.tile([P, CH], f32, name=f"vt{j}_{n}", tag="vt")
                    nc.sync.dma_start(out=vt[:], in_=v_r[n, :, sl])
                    head_tiles[(j, n)] = vt

            mx = spool.tile([1, 1], f32)
            nc.vector.reduce_max(mx[:], attn_sb[:], axis=mybir.AxisListType.X)
            ex = spool.tile([1, 4], f32)
            nc.vector.tensor_scalar(
                out=ex[:],
                in0=attn_sb[:],
                scalar1=mx[:],
                scalar2=None,
                op0=mybir.AluOpType.subtract,
            )
            nc.scalar.activation(ex[:], ex[:], mybir.ActivationFunctionType.Exp)
            sm = spool.tile([1, 1], f32)
            nc.vector.reduce_sum(sm[:], ex[:], axis=mybir.AxisListType.X)
            rec = spool.tile([1, 1], f32)
            nc.vector.reciprocal(rec[:], sm[:])
            w_sb = spool.tile([1, 4], f32)
            nc.vector.tensor_scalar(
                out=w_sb[:],
                in0=ex[:],
                scalar1=rec[:],
                scalar2=None,
                op0=mybir.AluOpType.mult,
            )

            ones_t = spool.tile([1, P], f32)
            nc.vector.memset(ones_t[:], 1.0)
            wb_ps = ppool.tile([P, 4], f32)
            nc.tensor.matmul(
                wb_ps[:], lhsT=ones_t[:], rhs=w_sb[:], start=True, stop=True
            )
            w_bc = spool.tile([P, 4], f32)  # w broadcast to all partitions
            nc.scalar.copy(w_bc[:], wb_ps[:])

            # ---- res = sum_n w[n] * v[n]
            # Per-chunk output DMAs go on the scalar-engine HWDGE ring so
            # writes overlap the reads streaming on the sync-engine ring.
            out_t = opool.tile([P, FREE], f32)
            res_r = res_d.rearrange("(p f) -> p f", p=P)
            for j in range(NCHUNK):
                sl = slice(j * CH, (j + 1) * CH)
                vts = []
                for n in range(N):
                    if (j, n) in head_tiles:
                        vts.append(head_tiles[(j, n)])
                        continue
                    vt = vpool.tile([P, CH], f32, name=f"vt{j}_{n}", tag="vt")
                    nc.sync.dma_start(out=vt[:], in_=v_r[n, :, sl])
                    vts.append(vt)
                osl = out_t[:, sl]
                # hybrid split: ACT does two multiplies (copy-with-scale),
                # DVE does two fused multiply-adds and the final add.
                m0 = tpool.tile([P, CH], f32, name=f"m0_{j}", tag="m0")
                m2 = tpool.tile([P, CH], f32, name=f"m2_{j}", tag="m2")
                nc.scalar.mul(m0[:], vts[0][:], w_bc[:, 0:1])
                nc.scalar.mul(m2[:], vts[2][:], w_bc[:, 2:3])
                nc.vector.scalar_tensor_tensor(
                    out=m0[:],
                    in0=vts[1][:],
                    scalar=w_bc[:, 1:2],
                    in1=m0[:],
                    op0=mybir.AluOpType.mult,
                    op1=mybir.AluOpType.add,
                )
                nc.vector.scalar_tensor_tensor(
                    out=m2[:],
                    in0=vts[3][:],
                    scalar=w_bc[:, 3:4],
                    in1=m2[:],
                    op0=mybir.AluOpType.mult,
                    op1=mybir.AluOpType.add,
                )
                nc.vector.tensor_add(osl, m0[:], m2[:])
            # output phase after all reads: split across both HWDGE rings
            half = FREE // 2
            nc.sync.dma_start(out=res_r[:, 0:half], in_=out_t[:, 0:half])
            nc.scalar.dma_start(out=res_r[:, half:], in_=out_t[:, half:])

    nc.compile()
    return nc


def get_nc():
    if "nc" not in _CACHE:
        _CACHE["nc"] = _build_nc()
    return _CACHE["nc"]


def make_in_maps(q, k, v):
    q = np.asarray(q, dtype=np.float32)
    k = np.asarray(k, dtype=np.float32)
    v = np.asarray(v, dtype=np.float32)
    in_maps = []
    for core in range(N_CORES):
        b, ch = divmod(core, 2)
        in_maps.append(
            {
                "q": np.ascontiguousarray(q[b, 0, 0]),
                "k": np.ascontiguousarray(k[b, 0]),
                "v": np.ascontiguousarray(
                    v[b, :, ch * CSH : (ch + 1) * CSH]
                ).reshape(N, FLAT),
            }
        )
    return in_maps


def assemble(results):
    res = np.empty((B, 1, C, HH, WW), dtype=np.float32)
    attn = np.empty((B, N), dtype=np.float32)
    for core in range(N_CORES):
        b, ch = divmod(core, 2)
        res[b, 0, ch * CSH : (ch + 1) * CSH] = results[core]["res"].reshape(
            CSH, HH, WW
        )
        if ch == 0:
            attn[b] = results[core]["attn"]
    return res, attn


def kernel(q, k, v):
    from concourse.bass_utils import run_bass_kernel_spmd

    nc = get_nc()
    in_maps = make_in_maps(q, k, v)
    out = run_bass_kernel_spmd(nc, in_maps, list(range(N_CORES)))
    return assemble(out.results)


# revision 14
# speedup vs baseline: 1.1993x; 1.1993x over previous
"""Trainium2 Bass kernel for a 4-network attention-weighted feature-map blend.

Math (per batch b):
    attn[b, n] = (q[b] / T) . k[b, n]                 (N=4, d=512)
    w = softmax(attn[b, :])
    res[b, c, y, x] = sum_n w[n] * v[b, n, c, y, x]   (C=64, 256x256)

Outputs: (res [4,1,64,256,256] f32, attn [4,4] f32)

Sharding: 8 cores = 4 batches x 2 halves of the C dimension. Each core gets
q/k for its batch (replicated) and a [4, 32, 256, 256] slice of v, flattened
to [4, 2097152]. The big work (weighted sum) is memory bound: 32 MiB in +
8 MiB out per core.

Per-core device program:
  - attn = q.k via 4 PSUM-accumulated matmuls (d split into 4x128 chunks)
  - softmax on one partition (reduce_max/sub/exp/reduce_sum/recip/mul)
  - broadcast w [1,4] -> [128,4] via K=1 outer-product matmul with ones
  - for each [128, 2048] tile: out = v0*w0; out = (vn*wn) + out  (DVE
    scalar_tensor_tensor chain), accumulating into a [128, 16384] out tile
  - one 8 MiB DMA of the out tile back to DRAM
"""

import numpy as np

TEMPERATURE = 22.627416997969522  # sqrt(512)

B = 4
N = 4
C = 64
HH = 256
WW = 256
D = 512

N_CORES = 8
CSH = C // 2  # 32 channels per core
FLAT = CSH * HH * WW  # 2097152 elements per (core, n)
P = 128
FREE = FLAT // P  # 16384 f32 per partition
# chunk widths (free dim) per DVE op / input DMA; tapered at the end so the
# final chunk's DVE chain (the serial tail after the last input lands) is short
CHUNKS = [2048] * 7 + [1024, 512, 512]
assert sum(CHUNKS) == FLAT // P

_CACHE = {}


def _build_nc():
    import concourse.bacc as bacc
    import concourse.mybir as mybir
    from concourse import tile

    f32 = mybir.dt.float32

    nc = bacc.Bacc(
        "TRN2",
        target_bir_lowering=False,
        debug=False,
        enable_asserts=False,
        num_devices=N_CORES,
    )

    q_d = nc.dram_tensor("q", [D], f32, kind="ExternalInput").ap()
    k_d = nc.dram_tensor("k", [N, D], f32, kind="ExternalInput").ap()
    v_d = nc.dram_tensor("v", [N, FLAT], f32, kind="ExternalInput").ap()
    res_d = nc.dram_tensor("res", [FLAT], f32, kind="ExternalOutput").ap()
    attn_d = nc.dram_tensor("attn", [N], f32, kind="ExternalOutput").ap()

    with tile.TileContext(nc) as tc:
        with (
            tc.tile_pool(name="small", bufs=1) as spool,
            tc.tile_pool(name="psum", bufs=1, space="PSUM") as ppool,
            tc.tile_pool(name="vin", bufs=8) as vpool,
            tc.tile_pool(name="tmp", bufs=3) as tpool,
            tc.tile_pool(name="outp", bufs=1) as opool,
        ):
            # ---- attn = (q/T) . k, then softmax -> w, broadcast to 128 parts
            # Tiny transposed loads go first on the sync ring (~1us of small
            # packets) so the softmax weights are ready early; fat v reads
            # stream right behind them.
            v_r = v_d.rearrange("n (p f) -> n p f", p=P)
            q_t = spool.tile([P, 4], f32)  # q_t[p, i] = q[i*128 + p]
            nc.sync.dma_start(out=q_t[:], in_=q_d.rearrange("(i p) -> p i", p=P))
            k_t = spool.tile([P, 4, 4], f32)  # k_t[p, i, n] = k[n, i*128 + p]
            for i in range(4):
                nc.sync.dma_start(
                    out=k_t[:, i, :],
                    in_=k_d[:, i * P : (i + 1) * P].rearrange("n p -> p n"),
                )

            attn_ps = ppool.tile([1, 4], f32)
            for i in range(4):
                nc.tensor.matmul(
                    attn_ps[:],
                    lhsT=q_t[:, i : i + 1],
                    rhs=k_t[:, i, :],
                    start=(i == 0),
                    stop=(i == 3),
                )

            attn_sb = spool.tile([1, 4], f32)
            nc.scalar.mul(attn_sb[:], attn_ps[:], 1.0 / TEMPERATURE)
            nc.gpsimd.dma_start(
                out=attn_d.rearrange("(p f) -> p f", p=1), in_=attn_sb[:]
            )



            mx = spool.tile([1, 1], f32)
            nc.vector.reduce_max(mx[:], attn_sb[:], axis=mybir.AxisListType.X)
            ex = spool.tile([1, 4], f32)
            nc.vector.tensor_scalar(
                out=ex[:],
                in0=attn_sb[:],
                scalar1=mx[:],
                scalar2=None,
                op0=mybir.AluOpType.subtract,
            )
            nc.scalar.activation(ex[:], ex[:], mybir.ActivationFunctionType.Exp)
            sm = spool.tile([1, 1], f32)
            nc.vector.reduce_sum(sm[:], ex[:], axis=mybir.AxisListType.X)
            rec = spool.tile([1, 1], f32)
            nc.vector.reciprocal(rec[:], sm[:])
            w_sb = spool.tile([1, 4], f32)
            nc.vector.tensor_scalar(
                out=w_sb[:],
                in0=ex[:],
                scalar1=rec[:],
                scalar2=None,
                op0=mybir.AluOpType.mult,
            )

            ones_t = spool.tile([1, P], f32)
            nc.vector.memset(ones_t[:], 1.0)
            wb_ps = ppool.tile([P, 4], f32)
            nc.tensor.matmul(
                wb_ps[:], lhsT=ones_t[:], rhs=w_sb[:], start=True, stop=True
            )
            w_bc = spool.tile([P, 4], f32)  # w broadcast to all partitions
            nc.scalar.copy(w_bc[:], wb_ps[:])

            # ---- res = sum_n w[n] * v[n]
            out_t = opool.tile([P, FREE], f32)
            res_r = res_d.rearrange("(p f) -> p f", p=P)
            off = 0
            for j, ch in enumerate(CHUNKS):
                sl = slice(off, off + ch)
                off += ch
                vts = []
                for n in range(N):
                    vt = vpool.tile(
                        [P, ch],
                        f32,
                        name=f"vt{j}_{n}",
                        tag="vt",
                        padded_shape=[P, max(CHUNKS)],
                    )
                    nc.sync.dma_start(out=vt[:], in_=v_r[n, :, sl])
                    vts.append(vt)
                osl = out_t[:, sl]
                nc.vector.tensor_scalar(
                    out=osl,
                    in0=vts[0][:],
                    scalar1=w_bc[:, 0:1],
                    scalar2=None,
                    op0=mybir.AluOpType.mult,
                )
                for n in range(1, N):
                    nc.vector.scalar_tensor_tensor(
                        out=osl,
                        in0=vts[n][:],
                        scalar=w_bc[:, n : n + 1],
                        in1=osl,
                        op0=mybir.AluOpType.mult,
                        op1=mybir.AluOpType.add,
                    )
            # output phase after all reads: split across both HWDGE rings
            half = FREE // 2
            nc.sync.dma_start(out=res_r[:, 0:half], in_=out_t[:, 0:half])
            nc.scalar.dma_start(out=res_r[:, half:], in_=out_t[:, half:])

    nc.compile()
    return nc


def get_nc():
    if "nc" not in _CACHE:
        _CACHE["nc"] = _build_nc()
    return _CACHE["nc"]


def make_in_maps(q, k, v):
    q = np.asarray(q, dtype=np.float32)
    k = np.asarray(k, dtype=np.float32)
    v = np.asarray(v, dtype=np.float32)
    in_maps = []
    for core in range(N_CORES):
        b, ch = divmod(core, 2)
        in_maps.append(
            {
                "q": np.ascontiguousarray(q[b, 0, 0]),
                "k": np.ascontiguousarray(k[b, 0]),
                "v": np.ascontiguousarray(
                    v[b, :, ch * CSH : (ch + 1) * CSH]
                ).reshape(N, FLAT),
            }
        )
    return in_maps


def assemble(results):
    res = np.empty((B, 1, C, HH, WW), dtype=np.float32)
    attn = np.empty((B, N), dtype=np.float32)
    for core in range(N_CORES):
        b, ch = divmod(core, 2)
        res[b, 0, ch * CSH : (ch + 1) * CSH] = results[core]["res"].reshape(
            CSH, HH, WW
        )
        if ch == 0:
            attn[b] = results[core]["attn"]
    return res, attn


def kernel(q, k, v):
    from concourse.bass_utils import run_bass_kernel_spmd

    nc = get_nc()
    in_maps = make_in_maps(q, k, v)
    out = run_bass_kernel_spmd(nc, in_maps, list(range(N_CORES)))
    return assemble(out.results)
